# revision 14
# baseline (speedup 1.0000x reference)
"""MAM dense kernel for Trainium2 (8 NeuronCores, SPMD data-parallel over M).

C[m,n] = max_k(x[m,k]*w[n,k]) + min_k(x[m,k]*w[n,k]) + bias[n]

Strategy per core (M_c = 512 rows of x):
  - Layout: n on partitions (8 tiles of 128 n's), k on the free axis.
  - Cast x and w to fp16 once (DVE 2x modes need 16-bit dtypes).
  - For each group of J=4 m-rows: broadcast those rows across all 128
    partitions via DMA, then on the Vector engine:
      q = w16 * x_bcast          (tensor_tensor mult, fp16 -> 2x mode)
      max tree: 10 rounds of pairwise tensor_tensor max over halves (2x)
      min tree: same with min
    Comparisons in fp16 are exact; only the fp32->fp16 casts and the
    product rounding lose precision (~1e-3 relative).
  - Combine max+min+bias in fp32, store transposed output [N, M_c];
    the host transposes back and concatenates core results.
"""

import os
import sys

sys.path.insert(0, "/opt/trn_rl_repo")

import numpy as np

M, K, N = 4096, 1024, 1024
N_CORES = 8
M_C = M // N_CORES  # 512 rows per core
NT = N // 128  # 8 n-tiles
J = 4  # m-rows per group
N_GROUPS = M_C // J  # 128 groups

_last_results = None  # BassKernelResults from the most recent run (for test.py)


def _build_nc(n_groups=N_GROUPS, nt=NT, j=J, k=K):
    import concourse.bass as bass
    import concourse.bacc as bacc
    import concourse.mybir as mybir
    import concourse.tile as tile
    from contextlib import ExitStack

    f32 = mybir.dt.float32
    f16 = mybir.dt.float16
    mult = mybir.AluOpType.mult
    amax = mybir.AluOpType.max
    amin = mybir.AluOpType.min
    aadd = mybir.AluOpType.add

    m_c = n_groups * j
    n_total = nt * 128

    nc = bacc.Bacc("TRN2", target_bir_lowering=False, debug=False)
    x_d = nc.dram_tensor("x", [m_c, k], f32, kind="ExternalInput").ap()
    w_d = nc.dram_tensor("w", [n_total, k], f32, kind="ExternalInput").ap()
    b_d = nc.dram_tensor("b", [n_total], f32, kind="ExternalInput").ap()
    o_d = nc.dram_tensor("o", [n_total, m_c], f32, kind="ExternalOutput").ap()
    x16_d = nc.dram_tensor("x16d", [m_c, k], f16).ap()  # fp16 scratch copy of x

    with tile.TileContext(nc) as tc, ExitStack() as ctx:
        p_const = ctx.enter_context(tc.tile_pool(name="const", bufs=1))

        # --- preamble: load + cast w and x to fp16 (no slot reuse: every
        # DMA here has at most one dependency — the direct2d DMA encoding
        # only supports a single sync wait).
        w16 = p_const.tile([128, nt, k], f16)
        b_sb = p_const.tile([128, nt], f32)
        out_sb = p_const.tile([128, nt, m_c], f32)
        with tc.tile_pool(name="stage", bufs=1) as p_stage:
            w32 = p_stage.tile([128, nt, k], f32)
            nc.sync.dma_start(w32[:], w_d.rearrange("(t p) k -> p t k", p=128))
            nc.vector.tensor_copy(w16[:], w32[:])

            x32 = p_stage.tile([128, j, k], f32)
            x16t = p_stage.tile([128, j, k], f16)
            nc.sync.dma_start(
                x32[:n_groups], x_d.rearrange("(p jj) k -> p jj k", jj=j)
            )
            nc.vector.tensor_copy(x16t[:n_groups], x32[:n_groups])
            nc.sync.dma_start(
                x16_d.rearrange("(p jj) k -> p jj k", jj=j), x16t[:n_groups]
            )

            nc.sync.dma_start(b_sb[:], b_d.rearrange("(t p) -> p t", p=128))

        p_xb = ctx.enter_context(tc.tile_pool(name="xb", bufs=3))
        p_q = ctx.enter_context(tc.tile_pool(name="q", bufs=1))
        p_a = ctx.enter_context(tc.tile_pool(name="ta", bufs=1))
        p_b = ctx.enter_context(tc.tile_pool(name="tb", bufs=1))
        p_r = ctx.enter_context(tc.tile_pool(name="r", bufs=2))

        w_b = w16[:].unsqueeze(2).broadcast_to([128, nt, j, k])

        for g in range(n_groups):
            # broadcast this group's j rows of x to all partitions (from DRAM)
            xb = p_xb.tile([128, j, k], f16)
            src = (
                x16_d[g * j : (g + 1) * j, :]
                .rearrange("j k -> (j k)")
                .unsqueeze(0)
                .broadcast_to([128, j * k])
            )
            nc.sync.dma_start(xb[:].rearrange("p j k -> p (j k)"), src)

            # products: q[p_n, t, jj, k] = w16[p_n, t, k] * x[g*j+jj, k]
            q = p_q.tile([128, nt, j, k], f16)
            xb_b = xb[:].unsqueeze(1).broadcast_to([128, nt, j, k])
            nc.vector.tensor_tensor(q[:], w_b, xb_b, mult)

            # pairwise-halves reduction trees (fp16, 2x mode)
            ta = p_a.tile([128, nt, j, k // 2], f16)
            tb = p_b.tile([128, nt, j, k // 4], f16)
            results = {}
            for op_name, op in (("mx", amax), ("mn", amin)):
                res = p_r.tile([128, nt, j], f32, tag=op_name)
                cur = q[:]
                f = k // 2
                use_a = True
                while f >= 1:
                    src0 = cur[:, :, :, 0:f]
                    src1 = cur[:, :, :, f : 2 * f]
                    if f == 1:
                        nc.vector.tensor_tensor(
                            res[:].unsqueeze(3), src0, src1, op
                        )
                    else:
                        dst = (ta if use_a else tb)[:, :, :, 0:f]
                        nc.vector.tensor_tensor(dst, src0, src1, op)
                        cur = dst
                        use_a = not use_a
                    f //= 2
                results[op_name] = res

            # combine: out[n, m] = max + min + bias[n]
            s = p_r.tile([128, nt, j], f32, tag="s")
            nc.vector.tensor_tensor(s[:], results["mx"][:], results["mn"][:], aadd)
            bias_b = b_sb[:].unsqueeze(2).broadcast_to([128, nt, j])
            nc.vector.tensor_tensor(
                out_sb[:, :, g * j : (g + 1) * j], s[:], bias_b, aadd
            )

        # --- store transposed output
        nc.sync.dma_start(o_d.rearrange("(t p) m -> p t m", p=128), out_sb[:])

    nc.compile()
    return nc


def kernel(x: np.ndarray, weight: np.ndarray, bias: np.ndarray) -> np.ndarray:
    global _last_results
    from concourse.bass_utils import run_bass_kernel_spmd

    x = np.ascontiguousarray(x, dtype=np.float32)
    weight = np.ascontiguousarray(weight, dtype=np.float32)
    bias = np.ascontiguousarray(bias, dtype=np.float32)

    nc = _build_nc()
    core_ids = list(range(N_CORES))
    in_maps = [
        {"x": x[c * M_C : (c + 1) * M_C], "w": weight, "b": bias}
        for c in core_ids
    ]
    res = run_bass_kernel_spmd(nc, in_maps, core_ids)
    _last_results = res

    out = np.empty((M, N), dtype=np.float32)
    for c in core_ids:
        out[c * M_C : (c + 1) * M_C, :] = res.results[c]["o"].T
    return out
